# revision 40
# baseline (speedup 1.0000x reference)
"""Trainium2 Bass kernel for nn_Luong_61684320305412 (bidirectional masked
softmax attention, B=8, L0=L1=2048, D=256).

Sharding: data-parallel over batch B across the 8 NeuronCores. Per core:

    S    = q0 @ q1^T                  [fp8e4 DoubleRow matmuls, K=256/instr]
    E    = exp(S/256), then masked entries forced to exactly 0 on DVE via
           one fused select per stripe: E = min(HUGE(1-m1[j]) + HUGE(1-m0[p]), E)
    E^T  = PE transpose of E tiles (fp16), packed 8/psum bank
    out0 = (E   @ [qn1 | 1])[:, 0:256] / col 256     (qn = q/16, fp16 chains)
    out1 = (E^T @ [qn0 | 1])[:, 0:256] / col 256

Key performance notes (measured on hw):
  - fp8e4 DoubleRow streams 2 rhs elems/cycle: a K=256,N=512 S-chunk costs
    ~216 ns; fp32r/fp16 would cost 2x. Tolerance (2e-2) comfortably absorbs
    the fp8 quantization of q (final rel err ~5e-3).
  - K=1 "mask row" matmuls are pathologically slow on the PE (~2.5x the cost
    of the K=256 data matmul), so the -inf mask outer-product is NOT added to
    the logits; masking runs as one scalar_tensor_tensor min-select on DVE.
    (Large negative exp args also NaN on the Act engine in fp16-out mode.)
  - E^T comes from PE transposes of E rather than recomputing S^T: the
    transposes are dependency-light PE filler and halve the Act exp load.
  - l1 is processed in two halves so the first half's out1 chains can start
    while the second half's S/exp still runs; engines are balanced with exp
    on Act, mask-select + psum drains on DVE, q casts split Act/DVE.
"""

import math
from contextlib import ExitStack

import numpy as np

import concourse.bass as bass
import concourse.tile as tile
from concourse import bacc, mybir
from concourse.bass_utils import run_bass_kernel_spmd

P = 128
B = 8
L = 2048          # L0 == L1
D = 256
T = L // P        # 16 row tiles
AUGW = D + 2      # 258: fp16 q/16 augmented with two ones columns
HUGE = 60000.0    # fp16-exact; mask select: min(HUGE*(1-m1[j]) + HUGE*(1-m0[p]), E)
                  # = 0 iff m0[p]=m1[j]=1 else E (E >= 0 always)
SCALE2 = 1.0 / 256.0   # applied to scores inside exp
INV16 = 1.0 / 16.0

f32 = mybir.dt.float32
f32r = mybir.dt.float32r
f16 = mybir.dt.float16
f8 = mybir.dt.float8e4
i32 = mybir.dt.int32
MUL = mybir.AluOpType.mult
EXP = mybir.ActivationFunctionType.Exp
DR = mybir.MatmulPerfMode.DoubleRow


def _emit(tc: tile.TileContext, ctx: ExitStack, io: dict):
    nc = tc.nc
    q0, q1, m0, m1 = io["q0"], io["q1"], io["mask0"], io["mask1"]
    out0, out1 = io["out0"], io["out1"]

    consts = ctx.enter_context(tc.tile_pool(name="consts", bufs=1))
    stage = ctx.enter_context(tc.tile_pool(name="stage", bufs=4))
    stage16 = ctx.enter_context(tc.tile_pool(name="stage16", bufs=6))
    qpool = ctx.enter_context(tc.tile_pool(name="qpool", bufs=1))
    e_pool = ctx.enter_context(tc.tile_pool(name="e", bufs=1))
    outp = ctx.enter_context(tc.tile_pool(name="outp", bufs=4))
    small = ctx.enter_context(tc.tile_pool(name="small", bufs=4))
    s_psum = ctx.enter_context(tc.tile_pool(name="s_psum", bufs=2, space="PSUM"))
    t_psum = ctx.enter_context(tc.tile_pool(name="t_psum", bufs=2, space="PSUM"))
    o_psum = ctx.enter_context(tc.tile_pool(name="o_psum", bufs=2, space="PSUM"))

    # ---- persistent operand tiles ----
    q0a = qpool.tile([P, T, AUGW], f16)   # q/16 | ones cols (out-matmul rhs)
    q1a = qpool.tile([P, T, AUGW], f16)
    q0t = qpool.tile([P, 2, L], f8)       # raw q, [d%128, d//128, l] DR layout
    q1t = qpool.tile([P, 2, L], f8)
    e0 = e_pool.tile([P, T, L], f16)      # E  [l0, l1]
    e1 = e_pool.tile([P, T, L], f16)      # E^T [l1, l0] (built by PE transpose)

    nc.vector.memset(q0a[:, :, D:AUGW], 1.0)
    nc.vector.memset(q1a[:, :, D:AUGW], 1.0)

    # ---- mask prep ----
    # row tile: wm1[0, j] = HUGE*(1-m1[j]), broadcast to all partitions by a
    # one-time K=1 PE outer product; col tile: s0[p, t] = HUGE*(1-m0[t*128+p])
    m1i = consts.tile([1, L], i32)
    nc.sync.dma_start(out=m1i, in_=m1.rearrange("(o l) -> o l", o=1))
    m1f = consts.tile([1, L], f32)
    nc.vector.tensor_copy(out=m1f, in_=m1i)
    wm1row = consts.tile([1, L], f16)
    nc.vector.tensor_scalar(out=wm1row, in0=m1f, scalar1=-HUGE, scalar2=HUGE,
                            op0=MUL, op1=mybir.AluOpType.add)
    onesrow = consts.tile([1, P], f16)
    nc.vector.memset(onesrow, 1.0)
    wm1 = consts.tile([P, L], f16)

    m0i = consts.tile([P, T], i32)
    nc.sync.dma_start(out=m0i, in_=m0.rearrange("(t p) -> p t", p=P))
    m0fc = consts.tile([P, T], f32)
    nc.vector.tensor_copy(out=m0fc, in_=m0i)
    s0 = consts.tile([P, T], f32)
    nc.vector.tensor_scalar(out=s0, in0=m0fc, scalar1=-HUGE, scalar2=HUGE,
                            op0=MUL, op1=mybir.AluOpType.add)

    from concourse.masks import make_identity
    ident_f = consts.tile([P, P], f32)
    make_identity(nc, ident_f)
    ident16 = consts.tile([P, P], f16)
    nc.vector.tensor_copy(out=ident16, in_=ident_f)

    # broadcast wm1row -> wm1 via PE outer product (one-time)
    for c in range(4):
        pw = s_psum.tile([P, 512], f32, tag="sp", name=f"pw{c}")
        nc.tensor.matmul(pw, lhsT=onesrow, rhs=wm1row[:, c * 512:(c + 1) * 512],
                         start=True, stop=True)
        nc.scalar.copy(wm1[:, c * 512:(c + 1) * 512], pw)

    # ---- load q, cast, and transpose into the fp8 DR layout ----
    def prep_pack(src, aug, tr, p4, on_act):
        pt = t_psum.tile([P, 1024], f16, tag="tp")
        for ti in range(4):
            t = p4 * 4 + ti
            st = stage.tile([P, D], f32, tag="st")
            nc.sync.dma_start(
                out=st, in_=src.rearrange("(t p) d -> t p d", p=P)[t]
            )
            st16 = stage16.tile([P, D], f16, tag="st16")
            if on_act:
                nc.scalar.mul(aug[:, t, 0:D], st, INV16)
                nc.scalar.copy(st16, st)
            else:
                nc.vector.tensor_scalar_mul(out=aug[:, t, 0:D], in0=st, scalar1=INV16)
                nc.vector.tensor_copy(out=st16, in_=st)
            for dc in range(2):
                nc.tensor.transpose(
                    pt[:, (ti * 2 + dc) * P:(ti * 2 + dc + 1) * P],
                    st16[:, dc * P:(dc + 1) * P], ident16,
                )
        dst = tr[:, :, p4 * 512:(p4 + 1) * 512]
        dstv = dst.rearrange("p two (t f) -> p t two f", t=4)
        srcv = pt.rearrange("p (t two f) -> p t two f", t=4, two=2)
        if on_act:
            nc.scalar.copy(dstv, srcv)
        else:
            nc.vector.tensor_copy(out=dstv, in_=srcv)

    # ---- S matmuls (pure DR, no accumulation) + exp + fused mask-select ----
    def s_half(t, H):
        ps = s_psum.tile([P, 1024], f32, tag="sp")
        for c in range(2):
            off = H * 1024 + c * 512
            nc.tensor.matmul(
                ps[:, c * 512:(c + 1) * 512],
                lhsT=q0t[:, :, t * P:(t + 1) * P],
                rhs=q1t[:, :, off:off + 512],
                start=True, stop=True, perf_mode=DR,
            )
        sl = slice(H * 1024, (H + 1) * 1024)
        nc.scalar.activation(out=e0[:, t, sl], in_=ps, func=EXP, scale=SCALE2)
        # masked entries -> exactly 0: e0 = min(wm1 + s0[p], e0)
        nc.vector.scalar_tensor_tensor(
            out=e0[:, t, sl], in0=wm1[:, sl], scalar=s0[:, t:t + 1],
            in1=e0[:, t, sl],
            op0=mybir.AluOpType.add, op1=mybir.AluOpType.min,
        )

    # ---- E^T tiles for source stripe i (consumable stripe-by-stripe) ----
    def et_half(i, half):
        pt = t_psum.tile([P, 1024], f16, tag="tp")
        for si in range(8):
            s = half * 8 + si
            nc.tensor.transpose(
                pt[:, si * P:(si + 1) * P],
                e0[:, i, s * P:(s + 1) * P], ident16,
            )
        dst = e1[:, half * 8:(half + 1) * 8, i * P:(i + 1) * P]
        nc.scalar.copy(dst, pt.rearrange("p (s f) -> p s f", s=8))

    # ---- one pairwise-interleaved pair of out accumulation chains ----
    def out_pair(esrc, raug, odram, j0, mul_act=False):
        pos = [o_psum.tile([P, AUGW], f32, tag="op", name=f"op{_k}") for _k in range(2)]
        for t in range(T):
            for k in range(2):
                j = j0 + k
                nc.tensor.matmul(
                    pos[k],
                    lhsT=esrc[:, t, j * P:(j + 1) * P],
                    rhs=raug[:, t, :],
                    start=(t == 0), stop=(t == T - 1),
                )
        for k in range(2):
            j = j0 + k
            rc = small.tile([P, 1], f32, tag="rc")
            nc.vector.reciprocal(rc, pos[k][:, D:D + 1])
            ot = outp.tile([P, D], f32, tag="ot")
            if mul_act:
                nc.scalar.mul(ot, pos[k][:, 0:D], rc)
            else:
                nc.vector.tensor_scalar_mul(out=ot, in0=pos[k][:, 0:D], scalar1=rc)
            nc.sync.dma_start(out=odram[j * P:(j + 1) * P, :], in_=ot)

    # ---- emission schedule ----
    # half-0 of S needs q1 stripes 0-7 + q0 stripe t: front-load those packs
    prep_pack(q1, q1a, q1t, 0, on_act=True)
    prep_pack(q0, q0a, q0t, 0, on_act=False)
    prep_pack(q1, q1a, q1t, 1, on_act=True)
    prep_pack(q0, q0a, q0t, 1, on_act=False)
    prep_pack(q1, q1a, q1t, 2, on_act=True)
    prep_pack(q0, q0a, q0t, 2, on_act=False)
    prep_pack(q1, q1a, q1t, 3, on_act=True)
    prep_pack(q0, q0a, q0t, 3, on_act=False)
    for t in range(T):
        s_half(t, 0)
    for t in range(T):
        s_half(t, 1)
        et_half(t, 0)
    for j0 in range(0, 8, 2):
        out_pair(e0, q0a, out1, j0)
    for t in range(T):
        et_half(t, 1)
        if t % 4 == 1:
            out_pair(e0, q0a, out1, 8 + (t // 4) * 2)
        if t % 2 == 1:
            out_pair(e1, q1a, out0, t - 1)

_CACHED_NC = None


def _build():
    global _CACHED_NC
    if _CACHED_NC is not None:
        return _CACHED_NC
    nc = bacc.Bacc("TRN2", target_bir_lowering=False, debug=False)
    io = {
        "q0": nc.dram_tensor("q0", [L, D], f32, kind="ExternalInput").ap(),
        "q1": nc.dram_tensor("q1", [L, D], f32, kind="ExternalInput").ap(),
        "mask0": nc.dram_tensor("mask0", [L], i32, kind="ExternalInput").ap(),
        "mask1": nc.dram_tensor("mask1", [L], i32, kind="ExternalInput").ap(),
        "out0": nc.dram_tensor("out0", [L, D], f32, kind="ExternalOutput").ap(),
        "out1": nc.dram_tensor("out1", [L, D], f32, kind="ExternalOutput").ap(),
    }
    with tile.TileContext(nc) as tc:
        with ExitStack() as ctx:
            _emit(tc, ctx, io)
    nc.compile()
    _CACHED_NC = nc
    return nc


def run_on_cores(q0, q1, mask0, mask1, trace=False):
    """Run the SPMD kernel; returns (out0, out1, BassKernelResults)."""
    nc = _build()
    in_maps = [
        {
            "q0": np.ascontiguousarray(q0[b], dtype=np.float32),
            "q1": np.ascontiguousarray(q1[b], dtype=np.float32),
            "mask0": np.ascontiguousarray(mask0[b], dtype=np.int32),
            "mask1": np.ascontiguousarray(mask1[b], dtype=np.int32),
        }
        for b in range(B)
    ]
    br = run_bass_kernel_spmd(nc, in_maps, list(range(B)), trace=trace)
    out0 = np.stack([br.results[b]["out0"] for b in range(B)])
    out1 = np.stack([br.results[b]["out1"] for b in range(B)])
    return out0, out1, br


def kernel(q0, q1, len0=None, len1=None, mask0=None, mask1=None, **_):
    q0 = np.asarray(q0, dtype=np.float32)
    q1 = np.asarray(q1, dtype=np.float32)
    mask0 = np.asarray(mask0, dtype=np.int32)
    mask1 = np.asarray(mask1, dtype=np.int32)
    out0, out1, _br = run_on_cores(q0, q1, mask0, mask1, trace=False)
    return out0, out1
